# revision 35
# baseline (speedup 1.0000x reference)
"""Trainium2 Bass kernel for DEIM multi-scale deformable attention.

Strategy (v2):
  - Data-parallel over batch: 16 batches -> 8 cores, 2 batches/core.
  - 600 (b,q) query slots per core in 5 tiles of <=128 partitions.
  - All NH*NP sampling locations for a (b,q,level) cluster within +-1 px of
    the shared reference point, so one 4x4-pixel x 256-channel window per
    (q,level) covers every bilinear corner (window start floor(ref)-1,
    clamped; exact grid_sample(zeros) reproduction — see hat weights).
  - The host pre-packs memory as bf16 "mem4": row r = the 4 rows
    [r, r+w, r+2w, r+3w] of the level grid concatenated (1024 values).
    One 8 KiB gather descriptor then fetches a whole 4x4x256 window
    (element = 4 consecutive mem4 rows = x0..x0+3), so a query tile needs
    ONE dma_gather of 512 descriptors for all 4 levels (vs 16x512 in v1):
    ~4x fewer descriptors to generate, 2x fewer HBM bytes (bf16).
    Window layout per query: win[q, l, (jx, iy, c)].
  - Per (q,l) the 16-pixel stencil M[(jx,iy),h] = sum_p attn*hatx*haty is
    built on DVE (prod/mm), broadcast-expanded over the 32 channels per
    head on ACT (bf16), and applied with a single stride-1 bf16
    tensor_mul (DVE 2x mode).
  - The 16-pixel + 4-level reduction runs on the TensorEngine as
    identity-matmul accumulation into PSUM (8 matmuls of N=512 per
    (tile,level), accumulated across levels), followed by one small DVE
    reduce of the 4 remaining pixel slots. This removes the big
    strided ADD reduces that dominated v1's Vector time.
  - Projections (offsets/attn logits) run in bf16 on the PE; the output
    projection stays f32 (transpose via PE identity + 3 matmuls).
"""

import os
from contextlib import ExitStack

import numpy as np

# ---------------------------------------------------------------------------
# Problem constants (hardcoded per harness contract)
# ---------------------------------------------------------------------------
B, Q, C, NH, NP, NL = 16, 300, 256, 8, 4, 4
HD = C // NH
SPATIAL = ((80, 80), (40, 40), (20, 20), (30, 70))  # (h, w) per level
S = sum(h * w for h, w in SPATIAL)  # 10500
BASE_L = [0, 6400, 8000, 8400]
H_L = [h for h, w in SPATIAL]
W_L = [w for h, w in SPATIAL]

NCORES = 8
BPC = B // NCORES          # batches per core
QS = BPC * Q               # query slots per core (600)
QT_SIZES = [128, 128, 128, 128, QS - 4 * 128]  # [128,128,128,128,88]
NQT = len(QT_SIZES)
MEMROWS = BPC * S          # 21000 window-anchor rows per core
WIN = 4                    # window size (pixels per axis)
ELEM = WIN * WIN * C       # gather element: 4x4 px x 256 ch = 4096 vals
ROWLEN = WIN * C           # mem4 row length (1024)

# per-level stencil-apply mode: 'act' = ACT broadcast-expands me to bf16,
# DVE multiplies at 2x; 'dve' = DVE multiplies directly with a broadcast
# AP at 1x (no expansion op)
EXPAND_ENG = ("gps", "act", "act", "act")
# identity-matmul reduce width (ISA caps matmul free size at 512 fp32)
RED_N = 512


def _build_program():
    import concourse.bacc as bacc
    import concourse.bass as bass
    import concourse.tile as tile
    from concourse import mybir
    from concourse.masks import make_identity

    f32 = mybir.dt.float32
    bf16 = mybir.dt.bfloat16
    i16 = mybir.dt.int16

    nc = bacc.Bacc("TRN2", target_bir_lowering=False, debug=False,
                   num_devices=NCORES)

    AF = mybir.ActivationFunctionType
    OP = mybir.AluOpType

    def ap_of(t, off, pairs):
        """Manual access pattern on a tile/AP: offset in elements relative
        to t's own offset; pairs = [[step, count], ...] (partition first;
        partition step rescaled to the tensor's per-partition stride)."""
        a = t[:] if hasattr(t, "__getitem__") else t
        pairs = [list(p) for p in pairs]
        if a.space in (bass.MemorySpace.SBUF, bass.MemorySpace.PSUM):
            pairs[0][0] *= a.ap[0][0]
        return bass.AP(tensor=a.tensor, offset=a.offset + off, ap=pairs)

    # ------------------------------------------------------------------
    # DRAM I/O
    # ------------------------------------------------------------------
    memd = nc.dram_tensor("mem4", [MEMROWS, ROWLEN], bf16, kind="ExternalInput")
    qTd = nc.dram_tensor("qT", [C, QS], bf16, kind="ExternalInput")
    # host-precomputed window geometry: pxm = refpix - window_start per
    # (padded query slot, l, xy); idxw = gather indices already wrapped in
    # the dma_gather [16-partition x replicated-x8] layout
    pxmd = nc.dram_tensor("pxm", [NQT * 128, 2 * NL], f32, kind="ExternalInput")
    idxwd = nc.dram_tensor("idxw", [128, NQT * 32], i16, kind="ExternalInput")
    woffd = nc.dram_tensor("Woff", [C, 256], bf16, kind="ExternalInput")
    wattnd = nc.dram_tensor("Wattn", [C, NH * NL * NP], bf16, kind="ExternalInput")
    woutd = nc.dram_tensor("Wout", [C, C], f32, kind="ExternalInput")
    boutd = nc.dram_tensor("bout", [1, C], f32, kind="ExternalInput")
    outd = nc.dram_tensor("out", [QS, C], f32, kind="ExternalOutput")

    with tile.TileContext(nc) as tc, ExitStack() as ctx:
        singles = ctx.enter_context(tc.tile_pool(name="singles", bufs=1))
        psum_mm = ctx.enter_context(tc.tile_pool(name="psum_mm", bufs=2, space="PSUM"))
        psum_red = ctx.enter_context(tc.tile_pool(name="psum_red", bufs=1, space="PSUM"))
        psum_tr = ctx.enter_context(tc.tile_pool(name="psum_tr", bufs=2, space="PSUM"))
        psum_o = ctx.enter_context(tc.tile_pool(name="psum_o", bufs=2, space="PSUM"))
        work = ctx.enter_context(tc.tile_pool(name="work", bufs=2))
        mepool = ctx.enter_context(tc.tile_pool(name="mepool", bufs=2))
        lvlp = ctx.enter_context(tc.tile_pool(name="lvlp", bufs=3))
        prodp = ctx.enter_context(tc.tile_pool(name="prodp", bufs=5))
        winp = ctx.enter_context(tc.tile_pool(name="winp", bufs=2))

        # ------- gather indices + window geometry: loaded, not computed ----
        idxw = singles.tile([128, NQT, 32], i16)
        nc.sync.dma_start(out=idxw, in_=idxwd.ap())
        pxm = singles.tile([128, NQT, 8], f32)
        nc.sync.dma_start(
            out=pxm,
            in_=pxmd.ap().rearrange("(t p) c -> p t c", p=128))

        # ------------- remaining one-time constants ------------------------
        sb_qT = singles.tile([128, 2, QS], bf16)
        nc.sync.dma_start(out=sb_qT, in_=qTd.ap().rearrange("(k p) q -> p k q", p=128))
        sb_Woff = singles.tile([128, 2, 256], bf16)
        nc.sync.dma_start(out=sb_Woff, in_=woffd.ap().rearrange("(k p) n -> p k n", p=128))
        sb_Wattn = singles.tile([128, 2, 128], bf16)
        nc.sync.dma_start(out=sb_Wattn, in_=wattnd.ap().rearrange("(k p) n -> p k n", p=128))
        sb_Wout = singles.tile([128, 2, 256], f32)
        nc.sync.dma_start(out=sb_Wout, in_=woutd.ap().rearrange("(k p) n -> p k n", p=128))
        sb_bout = singles.tile([1, 256], f32)
        nc.sync.dma_start(out=sb_bout, in_=boutd.ap())
        sb_ones = singles.tile([1, 128], f32)
        nc.vector.memset(sb_ones, 1.0)
        ident = singles.tile([128, 128], f32)
        make_identity(nc, ident[:])
        ident_b = singles.tile([128, 128], bf16)
        nc.vector.tensor_copy(ident_b[:, :], ident[:, :])
        jneg = singles.tile([128, WIN], f32)
        for j in range(WIN):
            nc.vector.memset(jneg[:, j:j + 1], float(-j))

        # ---------------- per query-tile pipeline ----------------
        # Software-pipelined: the "frontend" (gather kick-off, projections,
        # softmax, hats) of tile t+1 is emitted before the "backend" (level
        # loop, pixel reduce, output projection) of tile t, so each engine's
        # queue interleaves the two and the serial frontend chain hides
        # under the previous tile's level processing.

        def frontend(it):
            q0 = it * 128
            qlen = QT_SIZES[it]
            ql = slice(0, qlen)

            # one gather for all 4 levels: win[q, l, (jx, iy, c)]
            win = winp.tile([128, NL, ELEM], bf16, tag="win")
            nc.gpsimd.dma_gather(
                out_ap=win[:, :, :],
                in_ap=ap_of(memd.ap(), 0, [[ROWLEN, MEMROWS - (WIN - 1)], [1, ELEM]]),
                idxs_ap=idxw[:, it, :],
                num_idxs=512, num_idxs_reg=512,
                elem_size=ELEM, elem_step=ROWLEN)

            # PE projections: offs [q, (l,h,p,xy)], logits [q, (h,l,p)]
            ps_proj = psum_mm.tile([128, 384], f32, tag="ps_proj")
            ps_off = ps_proj[:, 0:256]
            ps_log = ps_proj[:, 256:384]
            nc.tensor.matmul(ps_off[ql, :], lhsT=sb_qT[:, 0, q0:q0 + qlen],
                             rhs=sb_Woff[:, 0, :], start=True, stop=False)
            nc.tensor.matmul(ps_off[ql, :], lhsT=sb_qT[:, 1, q0:q0 + qlen],
                             rhs=sb_Woff[:, 1, :], start=False, stop=True)
            nc.tensor.matmul(ps_log[ql, :], lhsT=sb_qT[:, 0, q0:q0 + qlen],
                             rhs=sb_Wattn[:, 0, :], start=True, stop=False)
            nc.tensor.matmul(ps_log[ql, :], lhsT=sb_qT[:, 1, q0:q0 + qlen],
                             rhs=sb_Wattn[:, 1, :], start=False, stop=True)

            offs = work.tile([128, 256], f32, tag="offs")
            nc.scalar.copy(offs[ql, :], ps_off[ql, :])

            # softmax over (l,p) per h; logits cols are (h,l,p)
            elog = work.tile([128, 128], f32, tag="elog")
            nc.scalar.activation(elog[ql, :], ps_log[ql, :], AF.Exp)
            ssum = work.tile([128, NH], f32, tag="ssum")
            nc.vector.tensor_reduce(ssum[ql, :],
                                    elog[ql, :].rearrange("q (h s) -> q h s", h=NH),
                                    axis=mybir.AxisListType.X, op=OP.add)
            rinv = work.tile([128, NH], f32, tag="rinv")
            nc.vector.reciprocal(rinv[ql, :], ssum[ql, :])
            # attnR[q, (l,h,p)] = elog[q, h,l,p] * rinv[q, h]
            attnR = work.tile([128, 128], f32, tag="attnR")
            nc.vector.tensor_mul(
                attnR[ql, :],
                ap_of(elog, 0, [[1, qlen], [4, NL], [16, NH], [1, NP]]),
                ap_of(rinv, 0, [[1, qlen], [0, NL], [1, NH], [0, NP]]),
            )

            # hats: U[q,l,xy,(h,p)] = offs + (refpix - window_start)
            uu = work.tile([128, NL, 2, 32], f32, tag="uu")
            nc.vector.tensor_add(
                uu[ql, :, :, :],
                ap_of(offs, 0, [[1, qlen], [64, NL], [1, 2], [2, 32]]),
                ap_of(pxm, it * 8, [[1, qlen], [2, NL], [1, 2], [0, 32]]))
            # A = |U - j| ; H = relu(1 - A)   layout [q, (j, l, xy, hp)]
            hat = work.tile([128, WIN, NL, 2, 32], f32, tag="hat")
            for j in range(WIN):
                nc.scalar.activation(hat[ql, j, :, :, :],
                                     uu[ql, :, :, :], AF.Abs,
                                     bias=jneg[ql, j:j + 1])
            nc.scalar.activation(hat[ql, :, :, :, :], hat[ql, :, :, :, :],
                                 AF.Relu, bias=1.0, scale=-1.0)

            # AFX[q, (l,h,p,jx)] = attnR[q,(l,h,p)] * hatx[q,(jx,l,hp)]
            afx = work.tile([128, NL, 8, NP, WIN], f32, tag="afx")
            nc.vector.tensor_mul(
                afx[ql, :, :, :, :],
                ap_of(hat, 0, [[1, qlen], [64, NL], [1, 32], [256, WIN]]),
                ap_of(attnR, 0, [[1, qlen], [32, NL], [1, 32], [0, WIN]]))
            return dict(win=win, hat=hat, afx=afx)

        def backend(it, fr):
            q0 = it * 128
            qlen = QT_SIZES[it]
            ql = slice(0, qlen)
            win, hat, afx = fr["win"], fr["hat"], fr["afx"]

            # per-level: stencil M + window multiply + PE accumulation
            ps_red = psum_red.tile([128, 1024], f32, tag="ps_red")
            prds = {}
            for l in range(NL):
                # prod[q, (jx,iy,h), p] = afx[q,(l,h,p,jx)] * haty[q,(iy,l,hp)]
                prod = lvlp.tile([128, 128, NP], f32, tag="prod")
                for p in range(NP):
                    nc.vector.tensor_mul(
                        ap_of(prod, p, [[1, qlen], [NP, 128]]),
                        ap_of(afx, l * 128 + p * WIN,
                              [[1, qlen], [1, WIN], [0, WIN], [16, 8]]),
                        ap_of(hat, l * 64 + 32 + p,
                              [[1, qlen], [0, WIN], [256, WIN], [4, 8]]))
                me_f = lvlp.tile([128, 128], f32, tag="me_f")
                nc.vector.tensor_reduce(me_f[ql, :], prod[ql, :, :],
                                        axis=mybir.AxisListType.X, op=OP.add)
                # apply stencil into a separate product tile (so PE reads of
                # level l never block the multiply of level l+1)
                prd = prodp.tile([128, ELEM], bf16, tag="prd")
                if EXPAND_ENG[l] == "act":
                    # ACT broadcast-expands me over the 32 ch/head (bf16),
                    # then the multiply runs stride-1 bf16 at DVE 2x rate
                    me_exp = mepool.tile([128, ELEM], bf16, tag="me_exp")
                    src = ap_of(me_f, 0, [[1, qlen], [8, 16], [1, 8], [0, 32]])
                    nc.scalar.copy(me_exp[ql, :], src)
                    nc.vector.tensor_mul(prd[ql, :], win[ql, l, :],
                                         me_exp[ql, :])
                else:
                    # direct broadcast multiply on DVE (1x) or GpSimd;
                    # saves the ACT expansion op
                    me_b = work.tile([128, 128], bf16, tag="me_b")
                    nc.vector.tensor_copy(me_b[ql, :], me_f[ql, :])
                    eng = nc.gpsimd if EXPAND_ENG[l] == "gps" else nc.vector
                    eng.tensor_mul(
                        ap_of(prd, 0, [[1, qlen], [256, 16], [32, 8], [1, 32]]),
                        ap_of(win, l * ELEM,
                              [[1, qlen], [256, 16], [32, 8], [1, 32]]),
                        ap_of(me_b, 0, [[1, qlen], [8, 16], [1, 8], [0, 32]]))
                prds[l] = prd

            # PE identity-matmul accumulation of the 16 pixel slots into
            # 1024 psum cols, accumulated across levels.  Level 0 (the slow
            # GpSimd product) goes LAST in the chain so the PE never stalls
            # on it mid-stream.
            mm_order = [1, 2, 3, 0]
            nmm = ELEM // RED_N
            for i, l in enumerate(mm_order):
                prd = prds[l]
                for b in range(nmm):
                    s = (b * RED_N) % 1024
                    nc.tensor.matmul(
                        ps_red[ql, s:s + RED_N],
                        lhsT=ident_b[:, ql],
                        rhs=prd[:, b * RED_N:(b + 1) * RED_N],
                        start=(i == 0 and b * RED_N < 1024),
                        stop=(i == len(mm_order) - 1
                              and (b + 1) * RED_N > ELEM - 1024))

            # stage-2: fold the 4 remaining pixel slots
            res = work.tile([128, 256], f32, tag="res")
            nc.vector.tensor_reduce(
                res[ql, :],
                ap_of(ps_red, 0, [[1, qlen], [1, 256], [256, 4]]),
                axis=mybir.AxisListType.X, op=OP.add)

            # output projection: out = res @ Wout + bout
            resT = work.tile([128, 2, 128], f32, tag="resT")
            ps_t = psum_tr.tile([128, 2, 128], f32, tag="ps_t")
            for hh in range(2):
                nc.tensor.transpose(ps_t[:, hh, ql],
                                    res[ql, 128 * hh:128 * (hh + 1)],
                                    ident[ql, ql])
                nc.scalar.copy(resT[:, hh, ql], ps_t[:, hh, ql])
            ps_out = psum_o.tile([128, 256], f32, tag="ps_out")
            nc.tensor.matmul(ps_out[ql, :], lhsT=resT[:, 0, ql],
                             rhs=sb_Wout[:, 0, :], start=True, stop=False)
            nc.tensor.matmul(ps_out[ql, :], lhsT=resT[:, 1, ql],
                             rhs=sb_Wout[:, 1, :], start=False, stop=False)
            nc.tensor.matmul(ps_out[ql, :], lhsT=sb_ones[0:1, ql],
                             rhs=sb_bout[0:1, :], start=False, stop=True)
            outt = work.tile([128, 256], f32, tag="outt")
            nc.scalar.copy(outt[ql, :], ps_out[ql, :])
            nc.sync.dma_start(out=outd.ap()[q0:q0 + qlen, :], in_=outt[ql, :])

        pending = {}
        for step in range(NQT + 1):
            if step < NQT:
                pending[step] = frontend(step)
            if step >= 1:
                backend(step - 1, pending.pop(step - 1))

    nc.compile()
    return nc


_NC_CACHE = {}
LAST_RESULTS = None


def _get_nc():
    if "nc" not in _NC_CACHE:
        _NC_CACHE["nc"] = _build_program()
    return _NC_CACHE["nc"]


def host_prep(query, memory, ref_points, W_off, b_off, W_attn, b_attn,
              W_out, b_out):
    """Build the 8 per-core input maps (pure layout transforms)."""
    import ml_dtypes
    bf16 = ml_dtypes.bfloat16

    query = np.ascontiguousarray(query, dtype=np.float32)
    memory = np.ascontiguousarray(memory, dtype=np.float32)
    ref = np.asarray(ref_points, dtype=np.float32)
    W_off = np.asarray(W_off, dtype=np.float32)
    b_off = np.asarray(b_off, dtype=np.float32)
    W_attn = np.asarray(W_attn, dtype=np.float32)
    b_attn = np.asarray(b_attn, dtype=np.float32)
    assert np.all(b_off == 0.0) and np.all(b_attn == 0.0), \
        "nonzero offset/attn biases not folded on device"
    # W_off cols (h,l,p,xy) -> (l,h,p,xy)
    Woff_r = np.ascontiguousarray(
        W_off.reshape(C, NH, NL, NP, 2).transpose(0, 2, 1, 3, 4).reshape(C, 256)
    ).astype(bf16)
    Wattn_r = np.ascontiguousarray(W_attn).astype(bf16)  # cols already (h,l,p)
    Wout = np.ascontiguousarray(W_out, dtype=np.float32)
    bout = np.ascontiguousarray(np.asarray(b_out, dtype=np.float32).reshape(1, C))

    # mem4: per (batch, level) rows r hold the 4 level rows r, r+w, r+2w,
    # r+3w concatenated (1024 ch), bf16.  Row indices keep the flat
    # [batch*S + BASE_L[l] + y*w + x] addressing of the original memory.
    mem_b = memory.astype(bf16)
    mem4 = np.empty((B, S, ROWLEN), dtype=bf16)
    for l, (h, w) in enumerate(SPATIAL):
        lo, hi = BASE_L[l], BASE_L[l] + h * w
        lvl = mem_b[:, lo:hi, :]  # [B, h*w, C]
        for k in range(WIN):
            mem4[:, lo:hi, k * C:(k + 1) * C] = np.roll(lvl, -k * w, axis=1)

    wh = np.array([[w, h] for h, w in SPATIAL], dtype=np.float32)
    whi = np.array([[w, h] for h, w in SPATIAL], dtype=np.int64)
    base = np.array(BASE_L, dtype=np.int64)
    wvec = whi[:, 0]
    in_maps = []
    for c in range(NCORES):
        bs = slice(BPC * c, BPC * (c + 1))
        qT = np.ascontiguousarray(
            query[bs].reshape(QS, C).T).astype(bf16)       # [256, 600]
        mem4c = np.ascontiguousarray(mem4[bs].reshape(MEMROWS, ROWLEN))
        refc = ref[bs].reshape(QS, NL, 2)
        refpix = refc * wh[None, :, :] - 0.5               # [600, l, xy]
        # window start (clamped) + gather row index, host-side
        xy0 = np.clip(np.floor(refpix).astype(np.int64) - 1, 0,
                      (whi - WIN)[None, :, :])              # [600, l, xy]
        batch = (np.arange(QS) // Q).astype(np.int64)
        idx = (batch[:, None] * S + base[None, :]
               + xy0[:, :, 1] * wvec[None, :] + xy0[:, :, 0])  # [600, l]
        pxm = (refpix - xy0).reshape(QS, 2 * NL).astype(np.float32)
        pxm = np.ascontiguousarray(
            np.concatenate([pxm,
                            np.zeros((NQT * 128 - QS, 2 * NL), np.float32)]))
        # wrap into the dma_gather index layout: per tile t the 512 indices
        # are ordered k = l*128 + p; stored at [p%16? -> row k%16, col k//16]
        # then replicated across the 8 Q7 partition groups.
        idx_pad = np.zeros((NQT * 128, NL), np.int64)
        idx_pad[:QS] = idx
        wrapped = np.empty((16, NQT, 32), np.int16)
        for t in range(NQT):
            flat = idx_pad[t * 128:(t + 1) * 128].T.reshape(512)  # k = l*128+p
            wrapped[:, t, :] = flat.reshape(32, 16).T.astype(np.int16)
        idxw = np.ascontiguousarray(
            np.tile(wrapped, (8, 1, 1)).reshape(128, NQT * 32))
        in_maps.append(dict(mem4=mem4c, qT=qT, pxm=pxm, idxw=idxw,
                            Woff=Woff_r, Wattn=Wattn_r, Wout=Wout, bout=bout))
    return in_maps


def kernel(**inputs):
    global LAST_RESULTS
    from concourse.bass_utils import run_bass_kernel_spmd

    nc = _get_nc()
    in_maps = host_prep(**inputs)
    trace = bool(int(os.environ.get("KERNEL_TRACE", "0")))
    res = run_bass_kernel_spmd(nc, in_maps, core_ids=list(range(NCORES)),
                               trace=trace)
    LAST_RESULTS = res
    out = np.empty((B, Q, C), dtype=np.float32)
    for c in range(NCORES):
        out[BPC * c:BPC * (c + 1)] = res.results[c]["out"].reshape(BPC, Q, C)
    return out


# revision 39
# speedup vs baseline: 1.3239x; 1.3239x over previous
"""Trainium2 Bass kernel for DEIM multi-scale deformable attention.

Strategy (v2):
  - Data-parallel over batch: 16 batches -> 8 cores, 2 batches/core.
  - 600 (b,q) query slots per core in 5 tiles of <=128 partitions.
  - All NH*NP sampling locations for a (b,q,level) cluster within +-1 px of
    the shared reference point, so one 4x4-pixel x 256-channel window per
    (q,level) covers every bilinear corner (window start floor(ref)-1,
    clamped; exact grid_sample(zeros) reproduction — see hat weights).
  - The host pre-packs memory as bf16 "mem4": row r = the 4 rows
    [r, r+w, r+2w, r+3w] of the level grid concatenated (1024 values).
    One 8 KiB gather descriptor then fetches a whole 4x4x256 window
    (element = 4 consecutive mem4 rows = x0..x0+3), so a query tile needs
    ONE dma_gather of 512 descriptors for all 4 levels (vs 16x512 in v1):
    ~4x fewer descriptors to generate, 2x fewer HBM bytes (bf16).
    Window layout per query: win[q, l, (jx, iy, c)].
  - Per (q,l) the 16-pixel stencil M[(jx,iy),h] = sum_p attn*hatx*haty is
    built on DVE (prod/mm), broadcast-expanded over the 32 channels per
    head on ACT (bf16), and applied with a single stride-1 bf16
    tensor_mul (DVE 2x mode).
  - The 16-pixel + 4-level reduction runs on the TensorEngine as
    identity-matmul accumulation into PSUM (8 matmuls of N=512 per
    (tile,level), accumulated across levels), followed by one small DVE
    reduce of the 4 remaining pixel slots. This removes the big
    strided ADD reduces that dominated v1's Vector time.
  - Projections (offsets/attn logits) run in bf16 on the PE; the output
    projection stays f32 (transpose via PE identity + 3 matmuls).
"""

import os
from contextlib import ExitStack

import numpy as np

# ---------------------------------------------------------------------------
# Problem constants (hardcoded per harness contract)
# ---------------------------------------------------------------------------
B, Q, C, NH, NP, NL = 16, 300, 256, 8, 4, 4
HD = C // NH
SPATIAL = ((80, 80), (40, 40), (20, 20), (30, 70))  # (h, w) per level
S = sum(h * w for h, w in SPATIAL)  # 10500
BASE_L = [0, 6400, 8000, 8400]
H_L = [h for h, w in SPATIAL]
W_L = [w for h, w in SPATIAL]

NCORES = 8
BPC = B // NCORES          # batches per core
QS = BPC * Q               # query slots per core (600)
QT_SIZES = [128, 128, 128, 128, QS - 4 * 128]  # [128,128,128,128,88]
NQT = len(QT_SIZES)
MEMROWS = BPC * S          # 21000 window-anchor rows per core
WIN = 4                    # window size (pixels per axis)
ELEM = WIN * WIN * C       # gather element: 4x4 px x 256 ch = 4096 vals
ROWLEN = WIN * C           # mem4 row length (1024)

# per-level stencil-apply mode: 'act' = ACT broadcast-expands me to bf16,
# DVE multiplies at 2x; 'dve' = DVE multiplies directly with a broadcast
# AP at 1x (no expansion op)
EXPAND_ENG = ("act", "act", "act", "act")
# identity-matmul reduce width (ISA caps matmul free size at 512 fp32)
RED_N = 512


def _build_program():
    import concourse.bacc as bacc
    import concourse.bass as bass
    import concourse.tile as tile
    from concourse import mybir
    from concourse.masks import make_identity

    f32 = mybir.dt.float32
    bf16 = mybir.dt.bfloat16
    i16 = mybir.dt.int16

    nc = bacc.Bacc("TRN2", target_bir_lowering=False, debug=False,
                   num_devices=NCORES)

    AF = mybir.ActivationFunctionType
    OP = mybir.AluOpType

    def ap_of(t, off, pairs):
        """Manual access pattern on a tile/AP: offset in elements relative
        to t's own offset; pairs = [[step, count], ...] (partition first;
        partition step rescaled to the tensor's per-partition stride)."""
        a = t[:] if hasattr(t, "__getitem__") else t
        pairs = [list(p) for p in pairs]
        if a.space in (bass.MemorySpace.SBUF, bass.MemorySpace.PSUM):
            pairs[0][0] *= a.ap[0][0]
        return bass.AP(tensor=a.tensor, offset=a.offset + off, ap=pairs)

    # ------------------------------------------------------------------
    # DRAM I/O
    # ------------------------------------------------------------------
    memd = nc.dram_tensor("mem4", [MEMROWS, ROWLEN], bf16, kind="ExternalInput")
    qTd = nc.dram_tensor("qT", [C, QS], bf16, kind="ExternalInput")
    # host-precomputed window geometry: pxm = refpix - window_start per
    # (padded query slot, l, xy); idxw = gather indices already wrapped in
    # the dma_gather [16-partition x replicated-x8] layout
    pxmd = nc.dram_tensor("pxm", [NQT * 128, 2 * NL], f32, kind="ExternalInput")
    idxwd = nc.dram_tensor("idxw", [128, NQT * 32], i16, kind="ExternalInput")
    woffd = nc.dram_tensor("Woff", [C, 256], bf16, kind="ExternalInput")
    wattnd = nc.dram_tensor("Wattn", [C, NH * NL * NP], bf16, kind="ExternalInput")
    woutd = nc.dram_tensor("Wout", [C, C], f32, kind="ExternalInput")
    boutd = nc.dram_tensor("bout", [1, C], f32, kind="ExternalInput")
    outd = nc.dram_tensor("out", [QS, C], f32, kind="ExternalOutput")

    with tile.TileContext(nc) as tc, ExitStack() as ctx:
        singles = ctx.enter_context(tc.tile_pool(name="singles", bufs=1))
        psum_mm = ctx.enter_context(tc.tile_pool(name="psum_mm", bufs=2, space="PSUM"))
        psum_red = ctx.enter_context(tc.tile_pool(name="psum_red", bufs=1, space="PSUM"))
        psum_tr = ctx.enter_context(tc.tile_pool(name="psum_tr", bufs=2, space="PSUM"))
        psum_o = ctx.enter_context(tc.tile_pool(name="psum_o", bufs=2, space="PSUM"))
        work = ctx.enter_context(tc.tile_pool(name="work", bufs=2))
        mepool = ctx.enter_context(tc.tile_pool(name="mepool", bufs=2))
        lvlp = ctx.enter_context(tc.tile_pool(name="lvlp", bufs=3))
        prodp = ctx.enter_context(tc.tile_pool(name="prodp", bufs=3))
        winp = ctx.enter_context(tc.tile_pool(name="winp", bufs=3))

        # ------- gather indices + window geometry: loaded, not computed ----
        idxw = singles.tile([128, NQT, 32], i16)
        nc.sync.dma_start(out=idxw, in_=idxwd.ap())
        pxm = singles.tile([128, NQT, 8], f32)
        nc.sync.dma_start(
            out=pxm,
            in_=pxmd.ap().rearrange("(t p) c -> p t c", p=128))

        # ------------- remaining one-time constants ------------------------
        sb_qT = singles.tile([128, 2, QS], bf16)
        nc.sync.dma_start(out=sb_qT, in_=qTd.ap().rearrange("(k p) q -> p k q", p=128))
        sb_Woff = singles.tile([128, 2, 256], bf16)
        nc.sync.dma_start(out=sb_Woff, in_=woffd.ap().rearrange("(k p) n -> p k n", p=128))
        sb_Wattn = singles.tile([128, 2, 128], bf16)
        nc.sync.dma_start(out=sb_Wattn, in_=wattnd.ap().rearrange("(k p) n -> p k n", p=128))
        sb_Wout = singles.tile([128, 2, 256], f32)
        nc.sync.dma_start(out=sb_Wout, in_=woutd.ap().rearrange("(k p) n -> p k n", p=128))
        sb_bout = singles.tile([1, 256], f32)
        nc.sync.dma_start(out=sb_bout, in_=boutd.ap())
        sb_ones = singles.tile([1, 128], f32)
        nc.vector.memset(sb_ones, 1.0)
        ident = singles.tile([128, 128], f32)
        make_identity(nc, ident[:])
        ident_b = singles.tile([128, 128], bf16)
        nc.vector.tensor_copy(ident_b[:, :], ident[:, :])
        jneg = singles.tile([128, WIN], f32)
        for j in range(WIN):
            nc.vector.memset(jneg[:, j:j + 1], float(-j))

        # ---------------- per query-tile pipeline ----------------
        # Software-pipelined: the "frontend" (gather kick-off, projections,
        # softmax, hats) of tile t+1 is emitted before the "backend" (level
        # loop, pixel reduce, output projection) of tile t, so each engine's
        # queue interleaves the two and the serial frontend chain hides
        # under the previous tile's level processing.

        def frontend(it):
            q0 = it * 128
            qlen = QT_SIZES[it]
            ql = slice(0, qlen)

            # one gather for all 4 levels: win[q, l, (jx, iy, c)]
            win = winp.tile([128, NL, ELEM], bf16, tag="win")
            nc.gpsimd.dma_gather(
                out_ap=win[:, :, :],
                in_ap=ap_of(memd.ap(), 0, [[ROWLEN, MEMROWS - (WIN - 1)], [1, ELEM]]),
                idxs_ap=idxw[:, it, :],
                num_idxs=512, num_idxs_reg=512,
                elem_size=ELEM, elem_step=ROWLEN)

            # PE projections: offs [q, (l,h,p,xy)], logits [q, (h,l,p)]
            ps_proj = psum_mm.tile([128, 384], f32, tag="ps_proj")
            ps_off = ps_proj[:, 0:256]
            ps_log = ps_proj[:, 256:384]
            nc.tensor.matmul(ps_off[ql, :], lhsT=sb_qT[:, 0, q0:q0 + qlen],
                             rhs=sb_Woff[:, 0, :], start=True, stop=False)
            nc.tensor.matmul(ps_off[ql, :], lhsT=sb_qT[:, 1, q0:q0 + qlen],
                             rhs=sb_Woff[:, 1, :], start=False, stop=True)
            nc.tensor.matmul(ps_log[ql, :], lhsT=sb_qT[:, 0, q0:q0 + qlen],
                             rhs=sb_Wattn[:, 0, :], start=True, stop=False)
            nc.tensor.matmul(ps_log[ql, :], lhsT=sb_qT[:, 1, q0:q0 + qlen],
                             rhs=sb_Wattn[:, 1, :], start=False, stop=True)

            offs = work.tile([128, 256], f32, tag="offs")
            nc.scalar.copy(offs[ql, :], ps_off[ql, :])

            # softmax over (l,p) per h; logits cols are (h,l,p)
            elog = work.tile([128, 128], f32, tag="elog")
            nc.scalar.activation(elog[ql, :], ps_log[ql, :], AF.Exp)
            ssum = work.tile([128, NH], f32, tag="ssum")
            nc.vector.tensor_reduce(ssum[ql, :],
                                    elog[ql, :].rearrange("q (h s) -> q h s", h=NH),
                                    axis=mybir.AxisListType.X, op=OP.add)
            rinv = work.tile([128, NH], f32, tag="rinv")
            nc.vector.reciprocal(rinv[ql, :], ssum[ql, :])
            # attnR[q, (l,h,p)] = elog[q, h,l,p] * rinv[q, h]
            attnR = work.tile([128, 128], f32, tag="attnR")
            nc.vector.tensor_mul(
                attnR[ql, :],
                ap_of(elog, 0, [[1, qlen], [4, NL], [16, NH], [1, NP]]),
                ap_of(rinv, 0, [[1, qlen], [0, NL], [1, NH], [0, NP]]),
            )

            # hats: U[q,l,xy,(h,p)] = offs + (refpix - window_start)
            uu = work.tile([128, NL, 2, 32], f32, tag="uu")
            nc.vector.tensor_add(
                uu[ql, :, :, :],
                ap_of(offs, 0, [[1, qlen], [64, NL], [1, 2], [2, 32]]),
                ap_of(pxm, it * 8, [[1, qlen], [2, NL], [1, 2], [0, 32]]))
            # A = |U - j| ; H = relu(1 - A)   layout [q, (j, l, xy, hp)]
            hat = work.tile([128, WIN, NL, 2, 32], f32, tag="hat")
            for j in range(WIN):
                nc.scalar.activation(hat[ql, j, :, :, :],
                                     uu[ql, :, :, :], AF.Abs,
                                     bias=jneg[ql, j:j + 1])
            nc.scalar.activation(hat[ql, :, :, :, :], hat[ql, :, :, :, :],
                                 AF.Relu, bias=1.0, scale=-1.0)

            # AFX[q, (l,h,p,jx)] = attnR[q,(l,h,p)] * hatx[q,(jx,l,hp)]
            afx = work.tile([128, NL, 8, NP, WIN], f32, tag="afx")
            nc.vector.tensor_mul(
                afx[ql, :, :, :, :],
                ap_of(hat, 0, [[1, qlen], [64, NL], [1, 32], [256, WIN]]),
                ap_of(attnR, 0, [[1, qlen], [32, NL], [1, 32], [0, WIN]]))
            return dict(win=win, hat=hat, afx=afx)

        def backend(it, fr):
            q0 = it * 128
            qlen = QT_SIZES[it]
            ql = slice(0, qlen)
            win, hat, afx = fr["win"], fr["hat"], fr["afx"]

            # per-level: stencil M + window multiply + PE accumulation
            ps_red = psum_red.tile([128, 1024], f32, tag="ps_red")
            prds = {}
            for l in range(NL):
                # prod[q, (jx,iy,h), p] = afx[q,(l,h,p,jx)] * haty[q,(iy,l,hp)]
                prod = lvlp.tile([128, 128, NP], f32, tag="prod")
                for p in range(NP):
                    nc.vector.tensor_mul(
                        ap_of(prod, p, [[1, qlen], [NP, 128]]),
                        ap_of(afx, l * 128 + p * WIN,
                              [[1, qlen], [1, WIN], [0, WIN], [16, 8]]),
                        ap_of(hat, l * 64 + 32 + p,
                              [[1, qlen], [0, WIN], [256, WIN], [4, 8]]))
                me_f = lvlp.tile([128, 128], f32, tag="me_f")
                nc.vector.tensor_reduce(me_f[ql, :], prod[ql, :, :],
                                        axis=mybir.AxisListType.X, op=OP.add)
                # apply stencil into a separate product tile (so PE reads of
                # level l never block the multiply of level l+1)
                prd = prodp.tile([128, ELEM], bf16, tag="prd")
                if EXPAND_ENG[l] == "act":
                    # ACT broadcast-expands me over the 32 ch/head (bf16),
                    # then the multiply runs stride-1 bf16 at DVE 2x rate
                    me_exp = mepool.tile([128, ELEM], bf16, tag="me_exp")
                    src = ap_of(me_f, 0, [[1, qlen], [8, 16], [1, 8], [0, 32]])
                    nc.scalar.copy(me_exp[ql, :], src)
                    nc.vector.tensor_mul(prd[ql, :], win[ql, l, :],
                                         me_exp[ql, :])
                else:
                    # direct broadcast multiply on DVE (1x) or GpSimd;
                    # saves the ACT expansion op
                    me_b = work.tile([128, 128], bf16, tag="me_b")
                    nc.vector.tensor_copy(me_b[ql, :], me_f[ql, :])
                    eng = nc.gpsimd if EXPAND_ENG[l] == "gps" else nc.vector
                    eng.tensor_mul(
                        ap_of(prd, 0, [[1, qlen], [256, 16], [32, 8], [1, 32]]),
                        ap_of(win, l * ELEM,
                              [[1, qlen], [256, 16], [32, 8], [1, 32]]),
                        ap_of(me_b, 0, [[1, qlen], [8, 16], [1, 8], [0, 32]]))
                prds[l] = prd

            # PE identity-matmul accumulation of the 16 pixel slots into
            # 1024 psum cols, accumulated across levels.  Level 0 (the slow
            # GpSimd product) goes LAST in the chain so the PE never stalls
            # on it mid-stream.
            mm_order = [1, 2, 3, 0] if "gps" in EXPAND_ENG else list(range(NL))
            nmm = ELEM // RED_N
            for i, l in enumerate(mm_order):
                prd = prds[l]
                for b in range(nmm):
                    s = (b * RED_N) % 1024
                    nc.tensor.matmul(
                        ps_red[ql, s:s + RED_N],
                        lhsT=ident_b[:, ql],
                        rhs=prd[:, b * RED_N:(b + 1) * RED_N],
                        start=(i == 0 and b * RED_N < 1024),
                        stop=(i == len(mm_order) - 1
                              and (b + 1) * RED_N > ELEM - 1024))

            # stage-2: fold the 4 remaining pixel slots
            res = work.tile([128, 256], f32, tag="res")
            nc.vector.tensor_reduce(
                res[ql, :],
                ap_of(ps_red, 0, [[1, qlen], [1, 256], [256, 4]]),
                axis=mybir.AxisListType.X, op=OP.add)

            # output projection: out = res @ Wout + bout
            resT = work.tile([128, 2, 128], f32, tag="resT")
            ps_t = psum_tr.tile([128, 2, 128], f32, tag="ps_t")
            for hh in range(2):
                nc.tensor.transpose(ps_t[:, hh, ql],
                                    res[ql, 128 * hh:128 * (hh + 1)],
                                    ident[ql, ql])
                nc.scalar.copy(resT[:, hh, ql], ps_t[:, hh, ql])
            ps_out = psum_o.tile([128, 256], f32, tag="ps_out")
            nc.tensor.matmul(ps_out[ql, :], lhsT=resT[:, 0, ql],
                             rhs=sb_Wout[:, 0, :], start=True, stop=False)
            nc.tensor.matmul(ps_out[ql, :], lhsT=resT[:, 1, ql],
                             rhs=sb_Wout[:, 1, :], start=False, stop=False)
            nc.tensor.matmul(ps_out[ql, :], lhsT=sb_ones[0:1, ql],
                             rhs=sb_bout[0:1, :], start=False, stop=True)
            outt = work.tile([128, 256], f32, tag="outt")
            nc.scalar.copy(outt[ql, :], ps_out[ql, :])
            nc.sync.dma_start(out=outd.ap()[q0:q0 + qlen, :], in_=outt[ql, :])

        pending = {}
        for step in range(NQT + 1):
            if step < NQT:
                pending[step] = frontend(step)
            if step >= 1:
                backend(step - 1, pending.pop(step - 1))

    nc.compile()
    return nc


_NC_CACHE = {}
LAST_RESULTS = None


def _get_nc():
    if "nc" not in _NC_CACHE:
        _NC_CACHE["nc"] = _build_program()
    return _NC_CACHE["nc"]


def host_prep(query, memory, ref_points, W_off, b_off, W_attn, b_attn,
              W_out, b_out):
    """Build the 8 per-core input maps (pure layout transforms)."""
    import ml_dtypes
    bf16 = ml_dtypes.bfloat16

    query = np.ascontiguousarray(query, dtype=np.float32)
    memory = np.ascontiguousarray(memory, dtype=np.float32)
    ref = np.asarray(ref_points, dtype=np.float32)
    W_off = np.asarray(W_off, dtype=np.float32)
    b_off = np.asarray(b_off, dtype=np.float32)
    W_attn = np.asarray(W_attn, dtype=np.float32)
    b_attn = np.asarray(b_attn, dtype=np.float32)
    assert np.all(b_off == 0.0) and np.all(b_attn == 0.0), \
        "nonzero offset/attn biases not folded on device"
    # W_off cols (h,l,p,xy) -> (l,h,p,xy)
    Woff_r = np.ascontiguousarray(
        W_off.reshape(C, NH, NL, NP, 2).transpose(0, 2, 1, 3, 4).reshape(C, 256)
    ).astype(bf16)
    Wattn_r = np.ascontiguousarray(W_attn).astype(bf16)  # cols already (h,l,p)
    Wout = np.ascontiguousarray(W_out, dtype=np.float32)
    bout = np.ascontiguousarray(np.asarray(b_out, dtype=np.float32).reshape(1, C))

    # mem4: per (batch, level) rows r hold the 4 level rows r, r+w, r+2w,
    # r+3w concatenated (1024 ch), bf16.  Row indices keep the flat
    # [batch*S + BASE_L[l] + y*w + x] addressing of the original memory.
    mem_b = memory.astype(bf16)
    mem4 = np.empty((B, S, ROWLEN), dtype=bf16)
    for l, (h, w) in enumerate(SPATIAL):
        lo, hi = BASE_L[l], BASE_L[l] + h * w
        lvl = mem_b[:, lo:hi, :]  # [B, h*w, C]
        for k in range(WIN):
            mem4[:, lo:hi, k * C:(k + 1) * C] = np.roll(lvl, -k * w, axis=1)

    wh = np.array([[w, h] for h, w in SPATIAL], dtype=np.float32)
    whi = np.array([[w, h] for h, w in SPATIAL], dtype=np.int64)
    base = np.array(BASE_L, dtype=np.int64)
    wvec = whi[:, 0]
    in_maps = []
    for c in range(NCORES):
        bs = slice(BPC * c, BPC * (c + 1))
        qT = np.ascontiguousarray(
            query[bs].reshape(QS, C).T).astype(bf16)       # [256, 600]
        mem4c = np.ascontiguousarray(mem4[bs].reshape(MEMROWS, ROWLEN))
        refc = ref[bs].reshape(QS, NL, 2)
        refpix = refc * wh[None, :, :] - 0.5               # [600, l, xy]
        # window start (clamped) + gather row index, host-side
        xy0 = np.clip(np.floor(refpix).astype(np.int64) - 1, 0,
                      (whi - WIN)[None, :, :])              # [600, l, xy]
        batch = (np.arange(QS) // Q).astype(np.int64)
        idx = (batch[:, None] * S + base[None, :]
               + xy0[:, :, 1] * wvec[None, :] + xy0[:, :, 0])  # [600, l]
        pxm = (refpix - xy0).reshape(QS, 2 * NL).astype(np.float32)
        pxm = np.ascontiguousarray(
            np.concatenate([pxm,
                            np.zeros((NQT * 128 - QS, 2 * NL), np.float32)]))
        # wrap into the dma_gather index layout: per tile t the 512 indices
        # are ordered k = l*128 + p; stored at [p%16? -> row k%16, col k//16]
        # then replicated across the 8 Q7 partition groups.
        idx_pad = np.zeros((NQT * 128, NL), np.int64)
        idx_pad[:QS] = idx
        wrapped = np.empty((16, NQT, 32), np.int16)
        for t in range(NQT):
            flat = idx_pad[t * 128:(t + 1) * 128].T.reshape(512)  # k = l*128+p
            wrapped[:, t, :] = flat.reshape(32, 16).T.astype(np.int16)
        idxw = np.ascontiguousarray(
            np.tile(wrapped, (8, 1, 1)).reshape(128, NQT * 32))
        in_maps.append(dict(mem4=mem4c, qT=qT, pxm=pxm, idxw=idxw,
                            Woff=Woff_r, Wattn=Wattn_r, Wout=Wout, bout=bout))
    return in_maps


def kernel(**inputs):
    global LAST_RESULTS
    from concourse.bass_utils import run_bass_kernel_spmd

    nc = _get_nc()
    in_maps = host_prep(**inputs)
    trace = bool(int(os.environ.get("KERNEL_TRACE", "0")))
    res = run_bass_kernel_spmd(nc, in_maps, core_ids=list(range(NCORES)),
                               trace=trace)
    LAST_RESULTS = res
    out = np.empty((B, Q, C), dtype=np.float32)
    for c in range(NCORES):
        out[BPC * c:BPC * (c + 1)] = res.results[c]["out"].reshape(BPC, Q, C)
    return out
